# revision 21
# baseline (speedup 1.0000x reference)
"""Block-diagonal 2x2 equalizer kernel for Trainium2 (8 NeuronCores), v4.

Per point (b, u, s, f) solves the 2x2 system M x = v by Cramer's rule:
    det = m00*m11 - m01*m10
    x0  = (m11*v0 - m01*v1) / det
    x1  = (m00*v1 - m10*v0) / det

Mixed precision (validated vs reference: rel err 3.3e-4, gate 2e-2):
  - det chain MUST be fp32: the data has near-singular blocks
    (min |det| = 1.5e-4 while |p0|,|p1| ~ 10); fp16 m-quantization alone
    perturbs det by ~1e-2 -> div-by-zero / garbage at those points.
  - numerators/output are fp16 (DVE 2x_1P mode): error there is
    relative-in-r, bounded by ~|x|*1e-3 even at singular points, and the
    gate is absolute (err.max()/|x|.max()).

Measured HW facts this version is built around (from NTFF traces):
  - DVE TT cadence: [896]fp32 1002ns, [1792]fp16-paired 1002ns,
    [896]fp16 535ns. Full speed also holds under concurrent DMA if the
    DVE isn't semaphore-starved.
  - Input DMA: ~18ns/descriptor gen on one hw queue, ~400GB/s byte cap.
    v3's 1024 descriptors made input DESC-bound (18.7us) -> v4 merges
    chunk-1 A and chunk-0 B strips into wide descriptors (704 total,
    byte-bound ~16us).
  - GPSIMD dma_start = software DGE at ~25GB/s — never use it. Stores
    go on the sync + scalar hardware queues as two 64-partition halves
    of one [128, 7168B-row] output tensor, issued only after the last
    DVE op (early stores steal input bandwidth).
  - ACT spline table load (1.5us) is hoisted off the critical path by a
    dummy 16-element reciprocal at program start.

Engine program (F = 896 cols per chunk, 2 chunks):
  DVE c0: PM0(p0) PM1(p1) DSUB(det) QA(q0,q1) QB(q3,q2) RS0(r0) FX0(x0)
          RS1(r1) FX1(x1)   [paired fp16 muls share the [v0|v1] window]
  DVE c1: PM0 PM1 DSUB QA RS0 FX0 QB RS1 FX1  <- only QB,RS1,FX1 depend
          on the last-arriving strip (hB1b): ~2.3us post-input tail.
  ACT:    dummy recip (table preload), recip c0, recip c1 (fp32->fp16)
  Input issue order (sync queue): A0a A0b B0 A1 B1a B1b.
"""

from contextlib import ExitStack

import numpy as np

import concourse.bass as bass
import concourse.mybir as mybir
from concourse.bass_utils import run_bass_kernel_spmd

# Problem shapes (hardcoded per contract)
B, U, A, NTX, T, S, F = 16, 4, 2, 1, 8, 14, 2048
SF = S * F               # 28672
NCORES = 8
BPC = B // NCORES        # 2 batches per core
QW = 448                 # inner width: SF = 64 * 448
ROWS = SF // QW          # 64 rows -> partition p = b*64 + row
COLS = U * QW            # 1792 free columns per plane
NCH = 2                  # chunks along the free axis
FD = COLS // NCH         # 896 free cols per chunk

TRACE = False
LAST_RESULTS = None


def _grid(x):
    """[BPC, U, SF] -> [128, COLS]; p = b*64 + sf//QW, col = u*QW + sf%QW."""
    return np.ascontiguousarray(
        x.reshape(BPC, U, ROWS, QW).transpose(0, 2, 1, 3).reshape(BPC * ROWS, COLS)
    )


def _ungrid(t):
    """Inverse of _grid: [128, COLS] -> [BPC, U, SF]."""
    return t.reshape(BPC, ROWS, U, QW).transpose(0, 2, 1, 3).reshape(BPC, U, SF)


def _build_nc():
    f32 = mybir.dt.float32
    f16 = mybir.dt.float16
    nc = bass.Bass("TRN2")
    # all strips as [128, 7168B]-descriptor transfers (measured optimum):
    # hA[k,0]=[m00|m11] f32, hA[k,1]=[m01|m10] f32
    # hBa[k]=[m11|m01|v0|v1] f16 (7168B rows), hBb[k]=[m10|m00] f16 (3584B rows)
    hA = nc.dram_tensor("hA", [NCH, 2, 128, 2 * FD], f32, kind="ExternalInput")
    hBa = nc.dram_tensor("hBa", [NCH, 128, 4 * FD], f16, kind="ExternalInput")
    hBb = nc.dram_tensor("hBb", [NCH, 128, 2 * FD], f16, kind="ExternalInput")
    xout = nc.dram_tensor("xout", [128, NCH * 2 * FD], f16, kind="ExternalOutput")
    dump = nc.dram_tensor("dump", [64, 128], f16, kind="ExternalOutput")

    with ExitStack() as ctx:
        tA = [ctx.enter_context(nc.sbuf_tensor(f"tA{k}", [128, 4 * FD], f32)) for k in range(NCH)]
        tB = [ctx.enter_context(nc.sbuf_tensor(f"tB{k}", [128, 6 * FD], f16)) for k in range(NCH)]
        tP = [ctx.enter_context(nc.sbuf_tensor(f"tP{k}", [128, 2 * FD], f32)) for k in range(NCH)]
        tQ = [ctx.enter_context(nc.sbuf_tensor(f"tQ{k}", [128, 4 * FD], f16)) for k in range(NCH)]
        tD = [ctx.enter_context(nc.sbuf_tensor(f"tD{k}", [128, FD], f32)) for k in range(NCH)]
        tW = [ctx.enter_context(nc.sbuf_tensor(f"tW{k}", [128, FD], f16)) for k in range(NCH)]
        tR = [ctx.enter_context(nc.sbuf_tensor(f"tR{k}", [128, 2 * FD], f16)) for k in range(NCH)]
        tX = ctx.enter_context(nc.sbuf_tensor("tX", [128, NCH * 2 * FD], f16))
        sA = [[ctx.enter_context(nc.semaphore(f"sA{k}{j}")) for j in range(2)] for k in range(NCH)]
        sBa = [ctx.enter_context(nc.semaphore(f"sBa{k}")) for k in range(NCH)]
        sBb = [ctx.enter_context(nc.semaphore(f"sBb{k}")) for k in range(NCH)]
        dve_sem = ctx.enter_context(nc.semaphore("dve_sem"))
        act_sem = ctx.enter_context(nc.semaphore("act_sem"))
        semO = ctx.enter_context(nc.semaphore("semO"))

        with nc.Block() as block:

            @block.sync
            def _(sync):
                # fp32 strips on the sync queue; fp16 strips ride the scalar
                # queue in parallel (aggregate DMA BW > one queue's rate, and
                # chunk-1's fp16 strips arrive ~3us earlier, cutting the tail)
                sync.dma_start(out=tA[0][:, : 2 * FD], in_=hA[0, 0]).then_inc(sA[0][0], 16)
                sync.dma_start(out=tA[0][:, 2 * FD :], in_=hA[0, 1]).then_inc(sA[0][1], 16)
                sync.dma_start(out=tA[1][:, : 2 * FD], in_=hA[1, 0]).then_inc(sA[1][0], 16)
                sync.dma_start(out=tA[1][:, 2 * FD :], in_=hA[1, 1]).then_inc(sA[1][1], 16)
                # store lower partition half once everything is computed
                sync.wait_ge(dve_sem, 9 * NCH)
                sync.dma_start(out=xout[0:64, :], in_=tX[0:64, :]).then_inc(semO, 16)
                sync.wait_ge(semO, 48)  # warmup dummy + 2 half stores

            # dve_sem: c0 ops 1..9, c1 ops 10..18
            @block.vector
            def _(vector):
                for k in range(NCH):
                    a, b, q, p, r = tA[k], tB[k], tQ[k], tP[k], tR[k]
                    x0 = tX[:, k * 2 * FD : k * 2 * FD + FD]
                    x1 = tX[:, k * 2 * FD + FD : (k + 1) * 2 * FD]
                    if k == 0:
                        # arrival order A0a, B0a, A0b, B0b
                        vector.wait_ge(sA[0][0], 16)
                        vector.tensor_mul(p[:, :FD], a[:, :FD], a[:, FD : 2 * FD]).then_inc(dve_sem, 1)   # PM0
                        vector.wait_ge(sBa[0], 16)
                        vector.tensor_mul(q[:, : 2 * FD], b[:, : 2 * FD], b[:, 2 * FD : 4 * FD]).then_inc(dve_sem, 1)  # QA
                        vector.wait_ge(sA[0][1], 16)
                        vector.tensor_mul(p[:, FD:], a[:, 2 * FD : 3 * FD], a[:, 3 * FD :]).then_inc(dve_sem, 1)  # PM1
                        vector.tensor_sub(tD[k][:], p[:, :FD], p[:, FD:]).then_inc(dve_sem, 1)            # DSUB
                        vector.wait_ge(sBb[0], 16)
                        vector.tensor_mul(q[:, 2 * FD :], b[:, 4 * FD :], b[:, 2 * FD : 4 * FD]).then_inc(dve_sem, 1)  # QB
                        vector.tensor_sub(r[:, :FD], q[:, :FD], q[:, FD : 2 * FD]).then_inc(dve_sem, 1)   # RS0
                        vector.wait_ge(act_sem, 2)
                        vector.tensor_mul(x0, r[:, :FD], tW[k][:]).then_inc(dve_sem, 1)                   # FX0
                        vector.tensor_sub(r[:, FD:], q[:, 3 * FD :], q[:, 2 * FD : 3 * FD]).then_inc(dve_sem, 1)  # RS1
                        vector.tensor_mul(x1, r[:, FD:], tW[k][:]).then_inc(dve_sem, 1)                   # FX1
                    else:
                        # arrival order A1a, A1b, B1a, B1b: det chain first;
                        # only QB,RS1,FX1 depend on the last strip
                        vector.wait_ge(sA[1][0], 16)
                        vector.tensor_mul(p[:, :FD], a[:, :FD], a[:, FD : 2 * FD]).then_inc(dve_sem, 1)   # PM0
                        vector.wait_ge(sA[1][1], 16)
                        vector.tensor_mul(p[:, FD:], a[:, 2 * FD : 3 * FD], a[:, 3 * FD :]).then_inc(dve_sem, 1)  # PM1
                        vector.tensor_sub(tD[k][:], p[:, :FD], p[:, FD:]).then_inc(dve_sem, 1)            # DSUB
                        vector.wait_ge(sBa[1], 16)
                        vector.tensor_mul(q[:, : 2 * FD], b[:, : 2 * FD], b[:, 2 * FD : 4 * FD]).then_inc(dve_sem, 1)  # QA
                        vector.tensor_sub(r[:, :FD], q[:, :FD], q[:, FD : 2 * FD]).then_inc(dve_sem, 1)   # RS0
                        vector.wait_ge(act_sem, 3)
                        vector.tensor_mul(x0, r[:, :FD], tW[k][:]).then_inc(dve_sem, 1)                   # FX0
                        vector.wait_ge(sBb[1], 16)
                        vector.tensor_mul(q[:, 2 * FD :], b[:, 4 * FD :], b[:, 2 * FD : 4 * FD]).then_inc(dve_sem, 1)  # QB
                        vector.tensor_sub(r[:, FD:], q[:, 3 * FD :], q[:, 2 * FD : 3 * FD]).then_inc(dve_sem, 1)  # RS1
                        vector.tensor_mul(x1, r[:, FD:], tW[k][:]).then_inc(dve_sem, 1)                   # FX1

            def _recip(eng, out_ap, in_ap):
                f32i = mybir.dt.float32
                return eng.add_instruction(
                    mybir.InstActivation(
                        name=nc.get_next_instruction_name(),
                        func=mybir.ActivationFunctionType.Reciprocal,
                        ins=[
                            eng.lower_ap(in_ap),
                            mybir.ImmediateValue(dtype=f32i, value=0.0),
                            mybir.ImmediateValue(dtype=f32i, value=1.0),
                            mybir.ImmediateValue(dtype=f32i, value=0.0),
                        ],
                        outs=[eng.lower_ap(out_ap)],
                    )
                )

            @block.scalar
            def _(scalar):
                # dummy recip: loads the ACT spline table off the critical path;
                # dummy store: warms up the scalar DMA queue's DGE (~3us cold)
                _recip(scalar, tW[0][:, 0:16], tD[0][:, 0:16]).then_inc(act_sem, 1)
                scalar.dma_start(out=tB[0][:, : 4 * FD], in_=hBa[0]).then_inc(sBa[0], 16)
                scalar.dma_start(out=tB[0][:, 4 * FD :], in_=hBb[0]).then_inc(sBb[0], 16)
                scalar.dma_start(out=tB[1][:, : 4 * FD], in_=hBa[1]).then_inc(sBa[1], 16)
                scalar.dma_start(out=tB[1][:, 4 * FD :], in_=hBb[1]).then_inc(sBb[1], 16)
                scalar.dma_start(out=dump[:], in_=tW[0][0:64, 0:128]).then_inc(semO, 16)
                for k, c in ((0, 4), (1, 12)):  # DSUB positions in dve_sem
                    scalar.wait_ge(dve_sem, c)
                    _recip(scalar, tW[k][:], tD[k][:]).then_inc(act_sem, 1)
                scalar.wait_ge(dve_sem, 9 * NCH)
                scalar.dma_start(out=xout[64:128, :], in_=tX[64:128, :]).then_inc(semO, 16)

    return nc


def make_in_maps(y, h, precoding_ind):
    """Host-side gather + dtype cast + strip packing. Returns per-core maps."""
    y = np.asarray(y, dtype=np.float32)
    h = np.asarray(h, dtype=np.float32)
    pi = np.asarray(precoding_ind).astype(np.int64)

    hg = h[:, pi[0]]                                     # [B, U, A, NTX, T, S, F]
    msel = np.stack(
        [hg[:, u, :, 0, 2 * u : 2 * u + 2] for u in range(U)], axis=1
    )                                                    # [B, U, A(i), 2(j), S, F]
    msel = np.ascontiguousarray(msel).reshape(B, U, 2, 2, SF)
    yr = np.ascontiguousarray(y).reshape(B, U, A, SF)

    in_maps = []
    for c in range(NCORES):
        b0 = c * BPC
        ms = msel[b0 : b0 + BPC]                         # [BPC, U, 2, 2, SF]
        ys = yr[b0 : b0 + BPC]                           # [BPC, U, A, SF]
        g32 = {
            name: _grid(ms[:, :, i, j])
            for name, (i, j) in {"m00": (0, 0), "m01": (0, 1), "m10": (1, 0), "m11": (1, 1)}.items()
        }
        v0g, v1g = _grid(ys[:, :, 0]), _grid(ys[:, :, 1])
        g16 = {n: a.astype(np.float16) for n, a in g32.items()}
        v0h, v1h = v0g.astype(np.float16), v1g.astype(np.float16)

        hA = np.empty((NCH, 2, 128, 2 * FD), np.float32)
        hBa = np.empty((NCH, 128, 4 * FD), np.float16)
        hBb = np.empty((NCH, 128, 2 * FD), np.float16)
        for k in range(NCH):
            s = slice(k * FD, (k + 1) * FD)
            hA[k, 0] = np.concatenate([g32["m00"][:, s], g32["m11"][:, s]], axis=1)
            hA[k, 1] = np.concatenate([g32["m01"][:, s], g32["m10"][:, s]], axis=1)
            hBa[k] = np.concatenate(
                [g16["m11"][:, s], g16["m01"][:, s], v0h[:, s], v1h[:, s]], axis=1
            )
            hBb[k] = np.concatenate([g16["m10"][:, s], g16["m00"][:, s]], axis=1)
        in_maps.append({
            "hA": np.ascontiguousarray(hA),
            "hBa": np.ascontiguousarray(hBa),
            "hBb": np.ascontiguousarray(hBb),
        })
    return in_maps


def assemble_output(results):
    """Per-core xout [128, NCH*2FD] f16 -> full [B, U, A, S, F] f32."""
    out = np.empty((B, U, A, S, F), np.float32)
    for c in range(NCORES):
        xo = np.asarray(results[c]["xout"]).astype(np.float32)
        x0 = np.empty((128, COLS), np.float32)
        x1 = np.empty((128, COLS), np.float32)
        for k in range(NCH):
            s = slice(k * FD, (k + 1) * FD)
            x0[:, s] = xo[:, k * 2 * FD : k * 2 * FD + FD]
            x1[:, s] = xo[:, k * 2 * FD + FD : (k + 1) * 2 * FD]
        out[c * BPC : (c + 1) * BPC, :, 0] = _ungrid(x0).reshape(BPC, U, S, F)
        out[c * BPC : (c + 1) * BPC, :, 1] = _ungrid(x1).reshape(BPC, U, S, F)
    return out


def kernel(y, h, precoding_ind):
    global LAST_RESULTS
    in_maps = make_in_maps(y, h, precoding_ind)
    nc = _build_nc()
    res = run_bass_kernel_spmd(nc, in_maps, list(range(NCORES)), trace=TRACE)
    LAST_RESULTS = res
    return assemble_output(res.results)


# revision 23
# speedup vs baseline: 1.0163x; 1.0163x over previous
"""Block-diagonal 2x2 equalizer kernel for Trainium2 (8 NeuronCores), v4.

Per point (b, u, s, f) solves the 2x2 system M x = v by Cramer's rule:
    det = m00*m11 - m01*m10
    x0  = (m11*v0 - m01*v1) / det
    x1  = (m00*v1 - m10*v0) / det

Mixed precision (validated vs reference: rel err 3.3e-4, gate 2e-2):
  - det chain MUST be fp32: the data has near-singular blocks
    (min |det| = 1.5e-4 while |p0|,|p1| ~ 10); fp16 m-quantization alone
    perturbs det by ~1e-2 -> div-by-zero / garbage at those points.
  - numerators/output are fp16 (DVE 2x_1P mode): error there is
    relative-in-r, bounded by ~|x|*1e-3 even at singular points, and the
    gate is absolute (err.max()/|x|.max()).

Measured HW facts this version is built around (from NTFF traces):
  - DVE TT cadence: [896]fp32 1002ns, [1792]fp16-paired 1002ns,
    [896]fp16 535ns. Full speed also holds under concurrent DMA if the
    DVE isn't semaphore-starved.
  - Input DMA: ~18ns/descriptor gen on one hw queue, ~400GB/s byte cap.
    v3's 1024 descriptors made input DESC-bound (18.7us) -> v4 merges
    chunk-1 A and chunk-0 B strips into wide descriptors (704 total,
    byte-bound ~16us).
  - GPSIMD dma_start = software DGE at ~25GB/s — never use it. Stores
    go on the sync + scalar hardware queues as two 64-partition halves
    of one [128, 7168B-row] output tensor, issued only after the last
    DVE op (early stores steal input bandwidth).
  - ACT spline table load (1.5us) is hoisted off the critical path by a
    dummy 16-element reciprocal at program start.

Engine program (F = 896 cols per chunk, 2 chunks):
  DVE c0: PM0(p0) PM1(p1) DSUB(det) QA(q0,q1) QB(q3,q2) RS0(r0) FX0(x0)
          RS1(r1) FX1(x1)   [paired fp16 muls share the [v0|v1] window]
  DVE c1: PM0 PM1 DSUB QA RS0 FX0 QB RS1 FX1  <- only QB,RS1,FX1 depend
          on the last-arriving strip (hB1b): ~2.3us post-input tail.
  ACT:    dummy recip (table preload), recip c0, recip c1 (fp32->fp16)
  Input issue order (sync queue): A0a A0b B0 A1 B1a B1b.
"""

from contextlib import ExitStack

import numpy as np

import concourse.bass as bass
import concourse.mybir as mybir
from concourse.bass_utils import run_bass_kernel_spmd

# Problem shapes (hardcoded per contract)
B, U, A, NTX, T, S, F = 16, 4, 2, 1, 8, 14, 2048
SF = S * F               # 28672
NCORES = 8
BPC = B // NCORES        # 2 batches per core
QW = 448                 # inner width: SF = 64 * 448
ROWS = SF // QW          # 64 rows -> partition p = b*64 + row
COLS = U * QW            # 1792 free columns per plane
NCH = 2                  # chunks along the free axis
FD = COLS // NCH         # 896 free cols per chunk

TRACE = False
LAST_RESULTS = None


def _grid(x):
    """[BPC, U, SF] -> [128, COLS]; p = b*64 + sf//QW, col = u*QW + sf%QW."""
    return np.ascontiguousarray(
        x.reshape(BPC, U, ROWS, QW).transpose(0, 2, 1, 3).reshape(BPC * ROWS, COLS)
    )


def _ungrid(t):
    """Inverse of _grid: [128, COLS] -> [BPC, U, SF]."""
    return t.reshape(BPC, ROWS, U, QW).transpose(0, 2, 1, 3).reshape(BPC, U, SF)


def _build_nc():
    f32 = mybir.dt.float32
    f16 = mybir.dt.float16
    nc = bass.Bass("TRN2")
    # all strips as [128, 7168B]-descriptor transfers (measured optimum):
    # hA[k,0]=[m00|m11] f32, hA[k,1]=[m01|m10] f32
    # hBa[k]=[m11|m01|v0|v1] f16 (7168B rows), hBb[k]=[m10|m00] f16 (3584B rows)
    hA = nc.dram_tensor("hA", [NCH, 2, 128, 2 * FD], f32, kind="ExternalInput")
    hBa = nc.dram_tensor("hBa", [NCH, 128, 4 * FD], f16, kind="ExternalInput")
    hBb = nc.dram_tensor("hBb", [NCH, 128, 2 * FD], f16, kind="ExternalInput")
    xout = nc.dram_tensor("xout", [128, NCH * 2 * FD], f16, kind="ExternalOutput")
    dump = nc.dram_tensor("dump", [64, 128], f16, kind="ExternalOutput")

    with ExitStack() as ctx:
        tA = [ctx.enter_context(nc.sbuf_tensor(f"tA{k}", [128, 4 * FD], f32)) for k in range(NCH)]
        tB = [ctx.enter_context(nc.sbuf_tensor(f"tB{k}", [128, 6 * FD], f16)) for k in range(NCH)]
        tP = [ctx.enter_context(nc.sbuf_tensor(f"tP{k}", [128, 2 * FD], f32)) for k in range(NCH)]
        tQ = [ctx.enter_context(nc.sbuf_tensor(f"tQ{k}", [128, 4 * FD], f16)) for k in range(NCH)]
        tD = [ctx.enter_context(nc.sbuf_tensor(f"tD{k}", [128, FD], f32)) for k in range(NCH)]
        tW = [ctx.enter_context(nc.sbuf_tensor(f"tW{k}", [128, FD], f16)) for k in range(NCH)]
        tR = [ctx.enter_context(nc.sbuf_tensor(f"tR{k}", [128, 2 * FD], f16)) for k in range(NCH)]
        tX = ctx.enter_context(nc.sbuf_tensor("tX", [128, NCH * 2 * FD], f16))
        sA = [[ctx.enter_context(nc.semaphore(f"sA{k}{j}")) for j in range(2)] for k in range(NCH)]
        sBa = [ctx.enter_context(nc.semaphore(f"sBa{k}")) for k in range(NCH)]
        sBb = [ctx.enter_context(nc.semaphore(f"sBb{k}")) for k in range(NCH)]
        dve_sem = ctx.enter_context(nc.semaphore("dve_sem"))
        act_sem = ctx.enter_context(nc.semaphore("act_sem"))
        semO = ctx.enter_context(nc.semaphore("semO"))

        with nc.Block() as block:

            @block.sync
            def _(sync):
                # one hw queue, issue order = DVE consumption order:
                # c0 interleaved fp32/fp16 (early QA); c1 det-chain first so
                # only QB,RS1,FX1 trail the last input strip (two-queue split
                # measured WORSE: fair-share starves the critical strips)
                sync.dma_start(out=tA[0][:, : 2 * FD], in_=hA[0, 0]).then_inc(sA[0][0], 16)
                sync.dma_start(out=tB[0][:, : 4 * FD], in_=hBa[0]).then_inc(sBa[0], 16)
                sync.dma_start(out=tA[0][:, 2 * FD :], in_=hA[0, 1]).then_inc(sA[0][1], 16)
                sync.dma_start(out=tB[0][:, 4 * FD :], in_=hBb[0]).then_inc(sBb[0], 16)
                sync.dma_start(out=tA[1][:, : 2 * FD], in_=hA[1, 0]).then_inc(sA[1][0], 16)
                sync.dma_start(out=tA[1][:, 2 * FD :], in_=hA[1, 1]).then_inc(sA[1][1], 16)
                sync.dma_start(out=tB[1][:, : 4 * FD], in_=hBa[1]).then_inc(sBa[1], 16)
                sync.dma_start(out=tB[1][:, 4 * FD :], in_=hBb[1]).then_inc(sBb[1], 16)
                # store lower partition half once everything is computed
                sync.wait_ge(dve_sem, 9 * NCH)
                sync.dma_start(out=xout[0:64, :], in_=tX[0:64, :]).then_inc(semO, 16)
                sync.wait_ge(semO, 48)  # warmup dummy + 2 half stores

            # dve_sem: c0 ops 1..9, c1 ops 10..18
            @block.vector
            def _(vector):
                for k in range(NCH):
                    a, b, q, p, r = tA[k], tB[k], tQ[k], tP[k], tR[k]
                    x0 = tX[:, k * 2 * FD : k * 2 * FD + FD]
                    x1 = tX[:, k * 2 * FD + FD : (k + 1) * 2 * FD]
                    if k == 0:
                        # arrival order A0a, B0a, A0b, B0b
                        vector.wait_ge(sA[0][0], 16)
                        vector.tensor_mul(p[:, :FD], a[:, :FD], a[:, FD : 2 * FD]).then_inc(dve_sem, 1)   # PM0
                        vector.wait_ge(sBa[0], 16)
                        vector.tensor_mul(q[:, : 2 * FD], b[:, : 2 * FD], b[:, 2 * FD : 4 * FD]).then_inc(dve_sem, 1)  # QA
                        vector.wait_ge(sA[0][1], 16)
                        vector.tensor_mul(p[:, FD:], a[:, 2 * FD : 3 * FD], a[:, 3 * FD :]).then_inc(dve_sem, 1)  # PM1
                        vector.tensor_sub(tD[k][:], p[:, :FD], p[:, FD:]).then_inc(dve_sem, 1)            # DSUB
                        vector.wait_ge(sBb[0], 16)
                        vector.tensor_mul(q[:, 2 * FD :], b[:, 4 * FD :], b[:, 2 * FD : 4 * FD]).then_inc(dve_sem, 1)  # QB
                        vector.tensor_sub(r[:, :FD], q[:, :FD], q[:, FD : 2 * FD]).then_inc(dve_sem, 1)   # RS0
                        vector.wait_ge(act_sem, 2)
                        vector.tensor_mul(x0, r[:, :FD], tW[k][:]).then_inc(dve_sem, 1)                   # FX0
                        vector.tensor_sub(r[:, FD:], q[:, 3 * FD :], q[:, 2 * FD : 3 * FD]).then_inc(dve_sem, 1)  # RS1
                        vector.tensor_mul(x1, r[:, FD:], tW[k][:]).then_inc(dve_sem, 1)                   # FX1
                    else:
                        # arrival order A1a, A1b, B1a, B1b: det chain first;
                        # only QB,RS1,FX1 depend on the last strip
                        vector.wait_ge(sA[1][0], 16)
                        vector.tensor_mul(p[:, :FD], a[:, :FD], a[:, FD : 2 * FD]).then_inc(dve_sem, 1)   # PM0
                        vector.wait_ge(sA[1][1], 16)
                        vector.tensor_mul(p[:, FD:], a[:, 2 * FD : 3 * FD], a[:, 3 * FD :]).then_inc(dve_sem, 1)  # PM1
                        vector.tensor_sub(tD[k][:], p[:, :FD], p[:, FD:]).then_inc(dve_sem, 1)            # DSUB
                        vector.wait_ge(sBa[1], 16)
                        vector.tensor_mul(q[:, : 2 * FD], b[:, : 2 * FD], b[:, 2 * FD : 4 * FD]).then_inc(dve_sem, 1)  # QA
                        vector.tensor_sub(r[:, :FD], q[:, :FD], q[:, FD : 2 * FD]).then_inc(dve_sem, 1)   # RS0
                        vector.wait_ge(act_sem, 3)
                        vector.tensor_mul(x0, r[:, :FD], tW[k][:]).then_inc(dve_sem, 1)                   # FX0
                        vector.wait_ge(sBb[1], 16)
                        vector.tensor_mul(q[:, 2 * FD :], b[:, 4 * FD :], b[:, 2 * FD : 4 * FD]).then_inc(dve_sem, 1)  # QB
                        vector.tensor_sub(r[:, FD:], q[:, 3 * FD :], q[:, 2 * FD : 3 * FD]).then_inc(dve_sem, 1)  # RS1
                        vector.tensor_mul(x1, r[:, FD:], tW[k][:]).then_inc(dve_sem, 1)                   # FX1

            def _recip(eng, out_ap, in_ap):
                f32i = mybir.dt.float32
                return eng.add_instruction(
                    mybir.InstActivation(
                        name=nc.get_next_instruction_name(),
                        func=mybir.ActivationFunctionType.Reciprocal,
                        ins=[
                            eng.lower_ap(in_ap),
                            mybir.ImmediateValue(dtype=f32i, value=0.0),
                            mybir.ImmediateValue(dtype=f32i, value=1.0),
                            mybir.ImmediateValue(dtype=f32i, value=0.0),
                        ],
                        outs=[eng.lower_ap(out_ap)],
                    )
                )

            @block.scalar
            def _(scalar):
                # dummy recip: loads the ACT spline table off the critical path;
                # dummy store: warms up the scalar DMA queue's DGE (~3us cold)
                _recip(scalar, tW[0][:, 0:16], tD[0][:, 0:16]).then_inc(act_sem, 1)
                scalar.dma_start(out=dump[0:2, 0:16], in_=tW[0][0:2, 0:16]).then_inc(semO, 16)
                for k, c in ((0, 4), (1, 12)):  # DSUB positions in dve_sem
                    scalar.wait_ge(dve_sem, c)
                    _recip(scalar, tW[k][:], tD[k][:]).then_inc(act_sem, 1)
                scalar.wait_ge(dve_sem, 9 * NCH)
                scalar.dma_start(out=xout[64:128, :], in_=tX[64:128, :]).then_inc(semO, 16)

    return nc


def make_in_maps(y, h, precoding_ind):
    """Host-side gather + dtype cast + strip packing. Returns per-core maps."""
    y = np.asarray(y, dtype=np.float32)
    h = np.asarray(h, dtype=np.float32)
    pi = np.asarray(precoding_ind).astype(np.int64)

    hg = h[:, pi[0]]                                     # [B, U, A, NTX, T, S, F]
    msel = np.stack(
        [hg[:, u, :, 0, 2 * u : 2 * u + 2] for u in range(U)], axis=1
    )                                                    # [B, U, A(i), 2(j), S, F]
    msel = np.ascontiguousarray(msel).reshape(B, U, 2, 2, SF)
    yr = np.ascontiguousarray(y).reshape(B, U, A, SF)

    in_maps = []
    for c in range(NCORES):
        b0 = c * BPC
        ms = msel[b0 : b0 + BPC]                         # [BPC, U, 2, 2, SF]
        ys = yr[b0 : b0 + BPC]                           # [BPC, U, A, SF]
        g32 = {
            name: _grid(ms[:, :, i, j])
            for name, (i, j) in {"m00": (0, 0), "m01": (0, 1), "m10": (1, 0), "m11": (1, 1)}.items()
        }
        v0g, v1g = _grid(ys[:, :, 0]), _grid(ys[:, :, 1])
        g16 = {n: a.astype(np.float16) for n, a in g32.items()}
        v0h, v1h = v0g.astype(np.float16), v1g.astype(np.float16)

        hA = np.empty((NCH, 2, 128, 2 * FD), np.float32)
        hBa = np.empty((NCH, 128, 4 * FD), np.float16)
        hBb = np.empty((NCH, 128, 2 * FD), np.float16)
        for k in range(NCH):
            s = slice(k * FD, (k + 1) * FD)
            hA[k, 0] = np.concatenate([g32["m00"][:, s], g32["m11"][:, s]], axis=1)
            hA[k, 1] = np.concatenate([g32["m01"][:, s], g32["m10"][:, s]], axis=1)
            hBa[k] = np.concatenate(
                [g16["m11"][:, s], g16["m01"][:, s], v0h[:, s], v1h[:, s]], axis=1
            )
            hBb[k] = np.concatenate([g16["m10"][:, s], g16["m00"][:, s]], axis=1)
        in_maps.append({
            "hA": np.ascontiguousarray(hA),
            "hBa": np.ascontiguousarray(hBa),
            "hBb": np.ascontiguousarray(hBb),
        })
    return in_maps


def assemble_output(results):
    """Per-core xout [128, NCH*2FD] f16 -> full [B, U, A, S, F] f32."""
    out = np.empty((B, U, A, S, F), np.float32)
    for c in range(NCORES):
        xo = np.asarray(results[c]["xout"]).astype(np.float32)
        x0 = np.empty((128, COLS), np.float32)
        x1 = np.empty((128, COLS), np.float32)
        for k in range(NCH):
            s = slice(k * FD, (k + 1) * FD)
            x0[:, s] = xo[:, k * 2 * FD : k * 2 * FD + FD]
            x1[:, s] = xo[:, k * 2 * FD + FD : (k + 1) * 2 * FD]
        out[c * BPC : (c + 1) * BPC, :, 0] = _ungrid(x0).reshape(BPC, U, S, F)
        out[c * BPC : (c + 1) * BPC, :, 1] = _ungrid(x1).reshape(BPC, U, S, F)
    return out


def kernel(y, h, precoding_ind):
    global LAST_RESULTS
    in_maps = make_in_maps(y, h, precoding_ind)
    nc = _build_nc()
    res = run_bass_kernel_spmd(nc, in_maps, list(range(NCORES)), trace=TRACE)
    LAST_RESULTS = res
    return assemble_output(res.results)


# revision 25
# speedup vs baseline: 1.2371x; 1.2173x over previous
"""Block-diagonal 2x2 equalizer kernel for Trainium2 (8 NeuronCores), v4.

Per point (b, u, s, f) solves the 2x2 system M x = v by Cramer's rule:
    det = m00*m11 - m01*m10
    x0  = (m11*v0 - m01*v1) / det
    x1  = (m00*v1 - m10*v0) / det

Mixed precision (validated vs reference: rel err 3.3e-4, gate 2e-2):
  - det chain MUST be fp32: the data has near-singular blocks
    (min |det| = 1.5e-4 while |p0|,|p1| ~ 10); fp16 m-quantization alone
    perturbs det by ~1e-2 -> div-by-zero / garbage at those points.
  - numerators/output are fp16 (DVE 2x_1P mode): error there is
    relative-in-r, bounded by ~|x|*1e-3 even at singular points, and the
    gate is absolute (err.max()/|x|.max()).

Measured HW facts this version is built around (from NTFF traces):
  - DVE TT cadence: [896]fp32 1002ns, [1792]fp16-paired 1002ns,
    [896]fp16 535ns. Full speed also holds under concurrent DMA if the
    DVE isn't semaphore-starved.
  - Input DMA: ~18ns/descriptor gen on one hw queue, ~400GB/s byte cap.
    v3's 1024 descriptors made input DESC-bound (18.7us) -> v4 merges
    chunk-1 A and chunk-0 B strips into wide descriptors (704 total,
    byte-bound ~16us).
  - GPSIMD dma_start = software DGE at ~25GB/s — never use it. Stores
    go on the sync + scalar hardware queues as two 64-partition halves
    of one [128, 7168B-row] output tensor, issued only after the last
    DVE op (early stores steal input bandwidth).
  - ACT spline table load (1.5us) is hoisted off the critical path by a
    dummy 16-element reciprocal at program start.

Engine program (F = 896 cols per chunk, 2 chunks):
  DVE c0: PM0(p0) PM1(p1) DSUB(det) QA(q0,q1) QB(q3,q2) RS0(r0) FX0(x0)
          RS1(r1) FX1(x1)   [paired fp16 muls share the [v0|v1] window]
  DVE c1: PM0 PM1 DSUB QA RS0 FX0 QB RS1 FX1  <- only QB,RS1,FX1 depend
          on the last-arriving strip (hB1b): ~2.3us post-input tail.
  ACT:    dummy recip (table preload), recip c0, recip c1 (fp32->fp16)
  Input issue order (sync queue): A0a A0b B0 A1 B1a B1b.
"""

from contextlib import ExitStack

import numpy as np

import concourse.bass as bass
import concourse.mybir as mybir
from concourse.bass_utils import run_bass_kernel_spmd

# Problem shapes (hardcoded per contract)
B, U, A, NTX, T, S, F = 16, 4, 2, 1, 8, 14, 2048
SF = S * F               # 28672
NCORES = 8
BPC = B // NCORES        # 2 batches per core
QW = 448                 # inner width: SF = 64 * 448
ROWS = SF // QW          # 64 rows -> partition p = b*64 + row
COLS = U * QW            # 1792 free columns per plane
NCH = 2                  # chunks along the free axis
FD = COLS // NCH         # 896 free cols per chunk

TRACE = False
LAST_RESULTS = None


def _grid(x):
    """[BPC, U, SF] -> [128, COLS]; p = b*64 + sf//QW, col = u*QW + sf%QW."""
    return np.ascontiguousarray(
        x.reshape(BPC, U, ROWS, QW).transpose(0, 2, 1, 3).reshape(BPC * ROWS, COLS)
    )


def _ungrid(t):
    """Inverse of _grid: [128, COLS] -> [BPC, U, SF]."""
    return t.reshape(BPC, ROWS, U, QW).transpose(0, 2, 1, 3).reshape(BPC, U, SF)


def _build_nc():
    f32 = mybir.dt.float32
    f16 = mybir.dt.float16
    nc = bass.Bass("TRN2")
    # all strips as [128, 7168B]-descriptor transfers (measured optimum):
    # hA[k,0]=[m00|m11] f32, hA[k,1]=[m01|m10] f32
    # hBa[k]=[m11|m01|v0|v1] f16 (7168B rows), hBb[k]=[m10|m00] f16 (3584B rows)
    hA = nc.dram_tensor("hA", [NCH, 2, 128, 2 * FD], f32, kind="ExternalInput")
    hBa = nc.dram_tensor("hBa", [NCH, 128, 4 * FD], f16, kind="ExternalInput")
    hBb = nc.dram_tensor("hBb", [NCH, 128, 2 * FD], f16, kind="ExternalInput")
    xout = nc.dram_tensor("xout", [128, NCH * 2 * FD], f16, kind="ExternalOutput")
    dump = nc.dram_tensor("dump", [64, 128], f16, kind="ExternalOutput")

    with ExitStack() as ctx:
        tA = [ctx.enter_context(nc.sbuf_tensor(f"tA{k}", [128, 4 * FD], f32)) for k in range(NCH)]
        tB = [ctx.enter_context(nc.sbuf_tensor(f"tB{k}", [128, 6 * FD], f16)) for k in range(NCH)]
        tP = [ctx.enter_context(nc.sbuf_tensor(f"tP{k}", [128, 2 * FD], f32)) for k in range(NCH)]
        tQ = [ctx.enter_context(nc.sbuf_tensor(f"tQ{k}", [128, 4 * FD], f16)) for k in range(NCH)]
        tD = [ctx.enter_context(nc.sbuf_tensor(f"tD{k}", [128, FD], f32)) for k in range(NCH)]
        tW = [ctx.enter_context(nc.sbuf_tensor(f"tW{k}", [128, FD], f16)) for k in range(NCH)]
        tR = [ctx.enter_context(nc.sbuf_tensor(f"tR{k}", [128, 2 * FD], f16)) for k in range(NCH)]
        tX = ctx.enter_context(nc.sbuf_tensor("tX", [128, NCH * 2 * FD], f16))
        sA = [[ctx.enter_context(nc.semaphore(f"sA{k}{j}")) for j in range(2)] for k in range(NCH)]
        sBa = [ctx.enter_context(nc.semaphore(f"sBa{k}")) for k in range(NCH)]
        sBb = [ctx.enter_context(nc.semaphore(f"sBb{k}")) for k in range(NCH)]
        dve_sem = ctx.enter_context(nc.semaphore("dve_sem"))
        act_sem = ctx.enter_context(nc.semaphore("act_sem"))
        semO = ctx.enter_context(nc.semaphore("semO"))

        with nc.Block() as block:

            @block.sync
            def _(sync):
                # one hw queue, issue order = DVE consumption order:
                # c0 interleaved fp32/fp16 (early QA); c1 det-chain first so
                # only QB,RS1,FX1 trail the last input strip (two-queue split
                # measured WORSE: fair-share starves the critical strips)
                sync.dma_start(out=tA[0][:, : 2 * FD], in_=hA[0, 0]).then_inc(sA[0][0], 16)
                sync.dma_start(out=tB[0][:, : 4 * FD], in_=hBa[0]).then_inc(sBa[0], 16)
                sync.dma_start(out=tA[0][:, 2 * FD :], in_=hA[0, 1]).then_inc(sA[0][1], 16)
                sync.dma_start(out=tB[0][:, 4 * FD :], in_=hBb[0]).then_inc(sBb[0], 16)
                sync.dma_start(out=tA[1][:, : 2 * FD], in_=hA[1, 0]).then_inc(sA[1][0], 16)
                sync.dma_start(out=tA[1][:, 2 * FD :], in_=hA[1, 1]).then_inc(sA[1][1], 16)
                sync.dma_start(out=tB[1][:, : 4 * FD], in_=hBa[1]).then_inc(sBa[1], 16)
                sync.dma_start(out=tB[1][:, 4 * FD :], in_=hBb[1]).then_inc(sBb[1], 16)
                # stores ride the same (warm) queue, FIFO behind the inputs;
                # store-c0's descriptor gen overlaps the last DVE ops
                sync.wait_ge(dve_sem, 9)
                sync.dma_start(out=xout[:, : 2 * FD], in_=tX[:, : 2 * FD]).then_inc(semO, 16)
                sync.wait_ge(dve_sem, 9 * NCH)
                sync.dma_start(out=xout[:, 2 * FD :], in_=tX[:, 2 * FD :]).then_inc(semO, 16)
                sync.wait_ge(semO, 48)  # warmup dummy + 2 chunk stores

            # dve_sem: c0 ops 1..9, c1 ops 10..18
            @block.vector
            def _(vector):
                for k in range(NCH):
                    a, b, q, p, r = tA[k], tB[k], tQ[k], tP[k], tR[k]
                    x0 = tX[:, k * 2 * FD : k * 2 * FD + FD]
                    x1 = tX[:, k * 2 * FD + FD : (k + 1) * 2 * FD]
                    if k == 0:
                        # arrival order A0a, B0a, A0b, B0b
                        vector.wait_ge(sA[0][0], 16)
                        vector.tensor_mul(p[:, :FD], a[:, :FD], a[:, FD : 2 * FD]).then_inc(dve_sem, 1)   # PM0
                        vector.wait_ge(sBa[0], 16)
                        vector.tensor_mul(q[:, : 2 * FD], b[:, : 2 * FD], b[:, 2 * FD : 4 * FD]).then_inc(dve_sem, 1)  # QA
                        vector.wait_ge(sA[0][1], 16)
                        vector.tensor_mul(p[:, FD:], a[:, 2 * FD : 3 * FD], a[:, 3 * FD :]).then_inc(dve_sem, 1)  # PM1
                        vector.tensor_sub(tD[k][:], p[:, :FD], p[:, FD:]).then_inc(dve_sem, 1)            # DSUB
                        vector.wait_ge(sBb[0], 16)
                        vector.tensor_mul(q[:, 2 * FD :], b[:, 4 * FD :], b[:, 2 * FD : 4 * FD]).then_inc(dve_sem, 1)  # QB
                        vector.tensor_sub(r[:, :FD], q[:, :FD], q[:, FD : 2 * FD]).then_inc(dve_sem, 1)   # RS0
                        vector.wait_ge(act_sem, 2)
                        vector.tensor_mul(x0, r[:, :FD], tW[k][:]).then_inc(dve_sem, 1)                   # FX0
                        vector.tensor_sub(r[:, FD:], q[:, 3 * FD :], q[:, 2 * FD : 3 * FD]).then_inc(dve_sem, 1)  # RS1
                        vector.tensor_mul(x1, r[:, FD:], tW[k][:]).then_inc(dve_sem, 1)                   # FX1
                    else:
                        # arrival order A1a, A1b, B1a, B1b: det chain first;
                        # only QB,RS1,FX1 depend on the last strip
                        vector.wait_ge(sA[1][0], 16)
                        vector.tensor_mul(p[:, :FD], a[:, :FD], a[:, FD : 2 * FD]).then_inc(dve_sem, 1)   # PM0
                        vector.wait_ge(sA[1][1], 16)
                        vector.tensor_mul(p[:, FD:], a[:, 2 * FD : 3 * FD], a[:, 3 * FD :]).then_inc(dve_sem, 1)  # PM1
                        vector.tensor_sub(tD[k][:], p[:, :FD], p[:, FD:]).then_inc(dve_sem, 1)            # DSUB
                        vector.wait_ge(sBa[1], 16)
                        vector.tensor_mul(q[:, : 2 * FD], b[:, : 2 * FD], b[:, 2 * FD : 4 * FD]).then_inc(dve_sem, 1)  # QA
                        vector.tensor_sub(r[:, :FD], q[:, :FD], q[:, FD : 2 * FD]).then_inc(dve_sem, 1)   # RS0
                        vector.wait_ge(act_sem, 3)
                        vector.tensor_mul(x0, r[:, :FD], tW[k][:]).then_inc(dve_sem, 1)                   # FX0
                        vector.wait_ge(sBb[1], 16)
                        vector.tensor_mul(q[:, 2 * FD :], b[:, 4 * FD :], b[:, 2 * FD : 4 * FD]).then_inc(dve_sem, 1)  # QB
                        vector.tensor_sub(r[:, FD:], q[:, 3 * FD :], q[:, 2 * FD : 3 * FD]).then_inc(dve_sem, 1)  # RS1
                        vector.tensor_mul(x1, r[:, FD:], tW[k][:]).then_inc(dve_sem, 1)                   # FX1

            def _recip(eng, out_ap, in_ap):
                f32i = mybir.dt.float32
                return eng.add_instruction(
                    mybir.InstActivation(
                        name=nc.get_next_instruction_name(),
                        func=mybir.ActivationFunctionType.Reciprocal,
                        ins=[
                            eng.lower_ap(in_ap),
                            mybir.ImmediateValue(dtype=f32i, value=0.0),
                            mybir.ImmediateValue(dtype=f32i, value=1.0),
                            mybir.ImmediateValue(dtype=f32i, value=0.0),
                        ],
                        outs=[eng.lower_ap(out_ap)],
                    )
                )

            @block.scalar
            def _(scalar):
                # dummy recip: loads the ACT spline table off the critical path;
                # dummy store: warms up the scalar DMA queue's DGE (~3us cold)
                _recip(scalar, tW[0][:, 0:16], tD[0][:, 0:16]).then_inc(act_sem, 1)
                scalar.dma_start(out=dump[0:2, 0:16], in_=tW[0][0:2, 0:16]).then_inc(semO, 16)
                for k, c in ((0, 4), (1, 12)):  # DSUB positions in dve_sem
                    scalar.wait_ge(dve_sem, c)
                    _recip(scalar, tW[k][:], tD[k][:]).then_inc(act_sem, 1)

    return nc


def make_in_maps(y, h, precoding_ind):
    """Host-side gather + dtype cast + strip packing. Returns per-core maps."""
    y = np.asarray(y, dtype=np.float32)
    h = np.asarray(h, dtype=np.float32)
    pi = np.asarray(precoding_ind).astype(np.int64)

    hg = h[:, pi[0]]                                     # [B, U, A, NTX, T, S, F]
    msel = np.stack(
        [hg[:, u, :, 0, 2 * u : 2 * u + 2] for u in range(U)], axis=1
    )                                                    # [B, U, A(i), 2(j), S, F]
    msel = np.ascontiguousarray(msel).reshape(B, U, 2, 2, SF)
    yr = np.ascontiguousarray(y).reshape(B, U, A, SF)

    in_maps = []
    for c in range(NCORES):
        b0 = c * BPC
        ms = msel[b0 : b0 + BPC]                         # [BPC, U, 2, 2, SF]
        ys = yr[b0 : b0 + BPC]                           # [BPC, U, A, SF]
        g32 = {
            name: _grid(ms[:, :, i, j])
            for name, (i, j) in {"m00": (0, 0), "m01": (0, 1), "m10": (1, 0), "m11": (1, 1)}.items()
        }
        v0g, v1g = _grid(ys[:, :, 0]), _grid(ys[:, :, 1])
        g16 = {n: a.astype(np.float16) for n, a in g32.items()}
        v0h, v1h = v0g.astype(np.float16), v1g.astype(np.float16)

        hA = np.empty((NCH, 2, 128, 2 * FD), np.float32)
        hBa = np.empty((NCH, 128, 4 * FD), np.float16)
        hBb = np.empty((NCH, 128, 2 * FD), np.float16)
        for k in range(NCH):
            s = slice(k * FD, (k + 1) * FD)
            hA[k, 0] = np.concatenate([g32["m00"][:, s], g32["m11"][:, s]], axis=1)
            hA[k, 1] = np.concatenate([g32["m01"][:, s], g32["m10"][:, s]], axis=1)
            hBa[k] = np.concatenate(
                [g16["m11"][:, s], g16["m01"][:, s], v0h[:, s], v1h[:, s]], axis=1
            )
            hBb[k] = np.concatenate([g16["m10"][:, s], g16["m00"][:, s]], axis=1)
        in_maps.append({
            "hA": np.ascontiguousarray(hA),
            "hBa": np.ascontiguousarray(hBa),
            "hBb": np.ascontiguousarray(hBb),
        })
    return in_maps


def assemble_output(results):
    """Per-core xout [128, NCH*2FD] f16 -> full [B, U, A, S, F] f32."""
    out = np.empty((B, U, A, S, F), np.float32)
    for c in range(NCORES):
        xo = np.asarray(results[c]["xout"]).astype(np.float32)
        x0 = np.empty((128, COLS), np.float32)
        x1 = np.empty((128, COLS), np.float32)
        for k in range(NCH):
            s = slice(k * FD, (k + 1) * FD)
            x0[:, s] = xo[:, k * 2 * FD : k * 2 * FD + FD]
            x1[:, s] = xo[:, k * 2 * FD + FD : (k + 1) * 2 * FD]
        out[c * BPC : (c + 1) * BPC, :, 0] = _ungrid(x0).reshape(BPC, U, S, F)
        out[c * BPC : (c + 1) * BPC, :, 1] = _ungrid(x1).reshape(BPC, U, S, F)
    return out


def kernel(y, h, precoding_ind):
    global LAST_RESULTS
    in_maps = make_in_maps(y, h, precoding_ind)
    nc = _build_nc()
    res = run_bass_kernel_spmd(nc, in_maps, list(range(NCORES)), trace=TRACE)
    LAST_RESULTS = res
    return assemble_output(res.results)
